# revision 1
# baseline (speedup 1.0000x reference)
"""Trainium2 Bass kernel for nn_ExampleEncoderLayer (dense transformer block).

Sharding: hybrid batch x sequence over 8 cores = 4 batches x 2 L-halves.
Per core (batch n, half): BN(x) -> h0 (full L, for K/V); Q + attention for
its 512-column window (inputs pre-rolled on host so the window is always
local columns [0,512)); out-projection + residual; the IbnNet conv stack on
its window. conv2's single cross-half halo column and the instance-norm
statistics are exchanged with two tiny pair-AllReduces.

v2: weights/activations in bf16 (same PE rate as f32r, half the HBM/SBUF
traffic); K/Q/V/exp attention operands in fp8e4 (raw exp(s) is O(1) so the
range fits; the whole attention branch contributes ~1.3% of the residual
so fp8's ~4% relative noise lands ~1e-4 on the output, far under the 2e-2
budget). The kernel front is software-pipelined per attention PAIR: the
softmax exp stream on the Activation engine (~75us, the real bottleneck of
the attention phase) starts ~17us in and hides under the K/Q/V/AV matmuls
instead of serializing after them. GpSimd drains the K/Q/V psums so the
DVE queue stays on the oT/den/residual path. The softmax 1/sqrt(d_model)
is applied as the exp ACTIVATE's scale constant so q/k stay at full scale
for fp8.
"""

import sys
import os

for _p in ("/opt/trn_rl_repo", "/root/.axon_site/_ro/trn_rl_repo"):
    if os.path.isdir(_p) and _p not in sys.path:
        sys.path.insert(0, _p)

import numpy as np
import ml_dtypes

E4 = ml_dtypes.float8_e4m3fn

import concourse.tile as tile
from concourse import bacc, mybir
from concourse import bass_utils

F32 = mybir.dt.float32
F32R = mybir.dt.float32r
BF16 = mybir.dt.bfloat16
FP8 = mybir.dt.float8e4
AF = mybir.ActivationFunctionType
ALU = mybir.AluOpType
AX = mybir.AxisListType
DR = mybir.MatmulPerfMode.DoubleRow

C = 1024      # d_model / channels / mid_channels
L = 1024      # sequence length
N_BATCH = 4
W = 512       # per-core L window
NT = C // 128  # 8 channel tiles
HEADS = 16
DH = 64
PAIRS = 8     # head pairs (2 heads = 128 partitions)
EPS = 1e-5
RG = [[0, 1], [2, 3], [4, 5], [6, 7]]  # core pairs sharing a batch

TRACE = False
LAST_RESULTS = None


def _build():
    from contextlib import ExitStack

    nc = bacc.Bacc("TRN2", target_bir_lowering=False, debug=False, num_devices=8)

    x_d = nc.dram_tensor("x", [C, L], BF16, kind="ExternalInput").ap()
    wqT_d = nc.dram_tensor("wqT", [C, C], FP8, kind="ExternalInput").ap()
    wkT_d = nc.dram_tensor("wkT", [C, C], FP8, kind="ExternalInput").ap()
    wvT_d = nc.dram_tensor("wvT", [C, C], FP8, kind="ExternalInput").ap()
    woT_d = nc.dram_tensor("woT", [C, C], FP8, kind="ExternalInput").ap()
    l1T_d = nc.dram_tensor("l1T", [C, C], BF16, kind="ExternalInput").ap()
    l2T_d = nc.dram_tensor("l2T", [3, C, C], FP8, kind="ExternalInput").ap()
    l3T_d = nc.dram_tensor("l3T", [C, C], BF16, kind="ExternalInput").ap()
    # packed per-channel columns: s0 t0 b1 b2 b3 (8 each) + mA mB
    vecs_d = nc.dram_tensor("vecs", [128, 43], F32, kind="ExternalInput").ap()
    # 2x128 selector for the denominator broadcast matmul:
    # row 0 = [1]*64+[0]*64, row 1 = [0]*64+[1]*64
    selm_d = nc.dram_tensor("selm", [2, 128], F32R, kind="ExternalInput").ap()
    out_d = nc.dram_tensor("out", [C, W // 2], F32, kind="ExternalOutput").ap()

    with tile.TileContext(nc) as tc:
      with (
        tc.tile_pool(name="pmisc", bufs=1) as pm,
        tc.tile_pool(name="pB", bufs=1) as pB,
        tc.tile_pool(name="dram", bufs=1, space="DRAM") as dp,
      ):
        vecs = pm.tile([128, 43], F32, tag="vecs")
        nc.scalar.dma_start(out=vecs[:], in_=vecs_d)
        s0 = vecs[:, 0:8]
        t0 = vecs[:, 8:16]
        b1 = vecs[:, 16:24]
        b2 = vecs[:, 24:32]
        b3 = vecs[:, 32:40]
        mA = vecs[:, 40:41]
        mB = vecs[:, 41:42]
        cinv = vecs[:, 42:43]

        def wdma(**kw):
            # all weight streams on the sync HWDGE queue: scalar is reserved
            # for ACT(exp) + x staging, gpsimd for psum drains + collectives
            nc.sync.dma_start(**kw)

        ones_f = pm.tile([128, 2], F32, tag="ones_f")
        nc.vector.memset(ones_f[:], 1.0)
        selm = pm.tile([2, 128], F32R, tag="selm")
        nc.sync.dma_start(out=selm[:], in_=selm_d)

        # conv-phase buffers (persist past the attention pool)
        h = [pB.tile([128, W], BF16, tag=f"h{i}", name=f"h{i}")
             for i in range(NT)]
        c1 = pB.tile([128, NT, C], BF16, tag="c1band")

        stA = ExitStack()
        pA = stA.enter_context(tc.tile_pool(name="pA", bufs=1))
        pE = stA.enter_context(tc.tile_pool(name="pexp", bufs=16))

        # h0 split: window half (lives through the residual) and far half
        # (only needed for K/V)
        h0a = pA.tile([128, NT, W], BF16, tag="h0a")
        h8a = pA.tile([128, NT, W], FP8, tag="h8a")
        h8b = pA.tile([128, NT, L - W], FP8, tag="h8b")
        v_sb = pA.tile([128, NT, HEADS, DH + 1], FP8, tag="v_sb")
        nc.vector.tensor_copy(
            out=v_sb[:, :, :, DH:DH + 1],
            in_=ones_f[:, 0:1].broadcast_to((128, NT * HEADS)).rearrange(
                "p (a h) -> p a h", a=NT).unsqueeze(3))
        kT = [pA.tile([128, L], FP8, tag=f"kT{i}", name=f"kT{i}")
              for i in range(PAIRS)]
        # Q^T padded per head-select: sel 0 keeps head-A rows 0:64 and zeroes
        # 64:128; sel 1 vice-versa. Scores then contract over the full K=128
        # so the PE HAM sees a fully-busy array (K=64 matmuls do not register
        # as busy and the clock would stay throttled at 4/8).
        qTp = [pA.tile([128, 2, W], FP8, tag=f"qTp{i}", name=f"qTp{i}")
               for i in range(PAIRS)]
        oT = [pA.tile([128, W], BF16, tag=f"oT{i}", name=f"oT{i}")
              for i in range(PAIRS)]
        o8 = pA.tile([128, PAIRS, W], FP8, tag="o8")

        def h8key(ct, khalf):
            # key-half view of BN(x), fp8: 0 -> window half, 1 -> far half
            return h8a[:, ct, :] if khalf == 0 else h8b[:, ct, :]

        def h8pair(a, khalf, kcols=None):
            t = h8a if khalf == 0 else h8b
            v = t[:, 2 * a:2 * a + 2, :]
            return v if kcols is None else v[:, :, kcols[0]:kcols[1]]

        # --- attention bookkeeping shared by the emission helpers ---
        expT = [None] * HEADS     # per-head exp tiles (pool pE)
        den2s = [None] * PAIRS
        dden = dp.tile([HEADS, W], F32, tag="dden")

        # PSUM pools, LIFO-ordered. Budget 8 banks of 2KB/partition:
        #   psO (AV, 2) + spsq (scores, 2) resident through attention
        #   + psA (K/Q, 3) during the QKV phases, psV (V, 3) during V,
        #   + psW (outproj 4) + dpsn (den bcast 2) afterwards.
        stS = ExitStack()
        spsq = stS.enter_context(tc.tile_pool(name="sc_ps", bufs=1, space="PSUM"))
        psO = None  # AV psum pool: opened after the merged K/Q/V phase

        # ---------------- emission helpers ----------------
        sunits = []   # pending (pr, hh, g) score+exp units

        def stage_scores(pr):
            for hh in range(2):
                for g in range(NT // 2):
                    sunits.append((pr, hh, g))

        def emit_sunit():
            if not sunits:
                return
            pr, hh, g = sunits.pop(0)
            head = 2 * pr + hh
            if g == 0:
                expT[head] = pE.tile([128, NT, W], FP8, tag="expT", name=f"expT{head}")
            sq = spsq.tile([128, 2, W], F32, tag="sq", name="sq")
            for j in range(2):
                kt = 2 * g + j
                nc.tensor.matmul(
                    sq[:, j, :], kT[pr][:, kt * 128:(kt + 1) * 128],
                    qTp[pr][:, hh, :])
            # one ACT call per 2 banks (the 352-cycle ACTIVATE overhead is
            # per instruction); the softmax /sqrt(d_model) rides the free
            # affine scale
            nc.scalar.activation(out=expT[head][:, 2 * g:2 * g + 2, :],
                                 in_=sq[:], func=AF.Exp, scale=1.0 / 32768.0)

        def emit_sunits(n):
            for _ in range(n):
                emit_sunit()

        def emit_av(head):
            # AV for one head; stash UNNORMALIZED o^T; denominator row (the
            # ones-column of v_sb) goes to partitions 0/1 of den2f via a
            # DRAM bounce (a partition move the DVE cannot do)
            pr, hh = divmod(head, 2)
            ops = psO.tile([DH + 1, W], F32, tag="po", name="avps")
            for kt in range(NT):
                nc.tensor.matmul(
                    ops[:], v_sb[:, kt, head, :], expT[head][:, kt, :],
                    start=(kt == 0), stop=(kt == NT - 1))
            lo, hi = hh * DH, (hh + 1) * DH
            nc.vector.tensor_copy(out=oT[pr][lo:hi, :], in_=ops[0:DH, :])
            denst = pm.tile([128, W], F32, tag="denst", bufs=2)
            nc.vector.tensor_copy(out=denst[DH:DH + 1, :],
                                  in_=ops[DH:DH + 1, :])
            nc.gpsimd.dma_start(out=dden[head:head + 1, :],
                                in_=denst[DH:DH + 1, :])
            if hh == 1:
                den2f = pm.tile([2, W], F32, tag="den2f", bufs=2)
                nc.gpsimd.dma_start(out=den2f[:],
                                    in_=dden[2 * pr:2 * pr + 2, :])
                den2r = pm.tile([2, W], F32, tag="den2r", bufs=2)
                nc.vector.reciprocal_approx_fast(out=den2r[:], in_=den2f[:])
                den2 = pm.tile([2, W], F32R, tag="den2", bufs=3)
                nc.vector.tensor_copy(out=den2[:], in_=den2r[:])
                den2s[pr] = den2

        avq = list(range(HEADS))  # heads whose AV is still pending

        def emit_avs(n):
            for _ in range(n):
                if avq:
                    emit_av(avq.pop(0))

        # ---------------- BN + per-pair K/Q, pipelined -------------------
        with tc.tile_pool(name="kq_ps", bufs=1, space="PSUM") as psA, \
             tc.tile_pool(name="v_ps", bufs=1, space="PSUM") as psV, \
             tc.tile_pool(name="wband", bufs=4) as wb, \
             tc.tile_pool(name="wbandv", bufs=2) as wbv, \
             tc.tile_pool(name="xstage", bufs=3) as xsp:
            # warm the PE clock (HAM) with throwaway matmuls while the x/
            # weight DMAs are in flight; ~3.4us of PE activity flips the
            # clock gate to 8/8 before the real work arrives
            wps = psA.tile([128, 2, W], F32, tag="kq", bufs=1)
            for i in range(60):
                nc.tensor.matmul(wps[:, 0, 0:128], selm[:], selm[:, 0:128],
                                 start=True, stop=True)
            # zero the dead halves of the padded Q
            for pr in range(PAIRS):
                nc.vector.memset(qTp[pr][DH:128, 0, :], 0.0)
                nc.vector.memset(qTp[pr][0:DH, 1, :], 0.0)

            # resident wk/wq; low halves first so pair 0 starts ASAP,
            # x tiles next, high halves after (contiguous half-DMAs hit
            # HBM line rate)
            kqK = wb.tile([128, NT // 2, 2, C], FP8, tag="kqK", bufs=1)
            kqQ = wb.tile([128, NT // 2, 2, C], FP8, tag="kqQ", bufs=1)
            wdma(out=kqK[:, :, :, 0:512],
                 in_=wkT_d[:, 0:512].rearrange(
                     "(a two p) c -> p a two c", two=2, p=128))
            wdma(out=kqQ[:, :, :, 0:512],
                 in_=wqT_d[:, 0:512].rearrange(
                     "(a two p) c -> p a two c", two=2, p=128))
            x_sbs = []
            for ct in range(NT):
                x_sb = xsp.tile([128, L], BF16, tag="xs")
                nc.sync.dma_start(out=x_sb[:],
                                  in_=x_d[ct * 128:(ct + 1) * 128, :])
                x_sbs.append(x_sb)
            wdma(out=kqK[:, :, :, 512:1024],
                 in_=wkT_d[:, 512:1024].rearrange(
                     "(a two p) c -> p a two c", two=2, p=128))
            wdma(out=kqQ[:, :, :, 512:1024],
                 in_=wqT_d[:, 512:1024].rearrange(
                     "(a two p) c -> p a two c", two=2, p=128))

            # V-projection work queue: one a-step (<=3 DR matmuls) is emitted
            # after each K/Q a-step of pairs 1..7, so the V accumulation
            # fills the exp-paced holes and keeps the PE duty high enough
            # for the HAM clock gate.
            vqueue = [(g, ci, chunk, a)
                      for g in range(2)
                      for ci, chunk in enumerate(((0, 1, 2), (3, 4, 5),
                                                  (6, 7)))
                      for a in range(NT // 2)]
            vstate = {"g": -1, "vb": None, "ps": None}

            def emit_vstep():
                if not vqueue:
                    return
                g, ci, chunk, a = vqueue.pop(0)
                if vstate["g"] != g:
                    vb = wbv.tile([128, NT // 2, 2, W], FP8, tag="vband",
                                  name=f"vb{g}")
                    wdma(out=vb[:],
                         in_=wvT_d[:, g * 512:(g + 1) * 512].rearrange(
                             "(a two p) c -> p a two c", two=2, p=128))
                    vstate["g"], vstate["vb"] = g, vb
                if a == 0:
                    vstate["ps"] = psV.tile([128, 3, W], F32, tag="vps",
                                            name=f"vps{g}{ci}")
                pss, vb = vstate["ps"], vstate["vb"]
                for i, kt in enumerate(chunk):
                    kh, kcol = divmod(kt * 128, W)
                    nc.tensor.matmul(
                        pss[:, i, :], h8pair(a, kh, (kcol, kcol + 128)),
                        vb[:, a, :, :],
                        start=(a == 0), stop=(a == NT // 2 - 1),
                        perf_mode=DR)
                if a == NT // 2 - 1:
                    for i, kt in enumerate(chunk):
                        nc.vector.tensor_copy(
                            out=v_sb[:, kt, g * 8:(g + 1) * 8, 0:DH],
                            in_=pss[:, i, :].rearrange(
                                "p (h d) -> p h d", h=8))
            for pr in range(PAIRS):
                kps = psA.tile([128, 2, W], F32, tag="kq", bufs=1)
                qps = psA.tile([128, W], F32, tag="q", bufs=1)
                for a in range(NT // 2):
                    if pr == 0:
                        # BN as the x tiles land (first pair only)
                        for ct in (2 * a, 2 * a + 1):
                            nc.vector.tensor_scalar(
                                out=h0a[:, ct, :], in0=x_sbs[ct][:, 0:W],
                                scalar1=s0[:, ct:ct + 1],
                                scalar2=t0[:, ct:ct + 1],
                                op0=ALU.mult, op1=ALU.add)
                            nc.vector.tensor_copy(out=h8a[:, ct, :],
                                                  in_=h0a[:, ct, :])
                            nc.vector.tensor_scalar(
                                out=h8b[:, ct, :], in0=x_sbs[ct][:, W:L],
                                scalar1=s0[:, ct:ct + 1],
                                scalar2=t0[:, ct:ct + 1],
                                op0=ALU.mult, op1=ALU.add)
                    for kh in range(2):
                        nc.tensor.matmul(
                            kps[:, kh, :],
                            kqK[:, a, :, pr * 128:(pr + 1) * 128],
                            h8pair(a, kh),
                            start=(a == 0), stop=(a == NT // 2 - 1),
                            perf_mode=DR)
                    nc.tensor.matmul(
                        qps[:], kqQ[:, a, :, pr * 128:(pr + 1) * 128],
                        h8pair(a, 0),
                        start=(a == 0), stop=(a == NT // 2 - 1),
                        perf_mode=DR)
                    # score units of the previous pair + one V a-step
                    # between K/Q steps: paces the ACT exp stream and keeps
                    # the PE busy while it waits on exp
                    if pr >= 1:
                        emit_sunit()
                        emit_vstep()
                        emit_sunit()
                # drains on gpsimd so the DVE stays free for the oT path;
                # the PE chews queued score units while they run
                nc.vector.tensor_copy(
                    out=kT[pr][:].rearrange("p (a w) -> p a w", a=2),
                    in_=kps[:])
                nc.vector.tensor_copy(out=qTp[pr][0:DH, 0, :],
                                      in_=qps[0:DH, :])
                nc.vector.tensor_copy(out=qTp[pr][DH:128, 1, :],
                                      in_=qps[DH:128, :])
                stage_scores(pr)

        # drain any V steps not absorbed by the pair phases
        while vqueue:
            emit_vstep()

        # throwaway pair-AllReduce: synchronizes the core pair early so the
        # halo AllReduce later does not pay the accumulated trigger skew
        cc0i = dp.tile([128, 1], F32, tag="cc0i")
        cc0o = dp.tile([128, 1], F32, tag="cc0o")
        nc.sync.dma_start(out=cc0i[:], in_=ones_f[:, 0:1])
        nc.gpsimd.collective_compute(
            "AllReduce", ALU.add, replica_groups=RG,
            ins=[cc0i[:].opt()], outs=[cc0o[:].opt()])

        # outproj wo bands (fp8 pair layout), prefetched now
        obs = []
        for gi in range(2):
            ob = pA.tile([128, NT // 2, 2, W], FP8,
                         tag=f"oband{gi}", name=f"oband{gi}")
            wdma(out=ob[:],
                 in_=woT_d[:, gi * 512:(gi + 1) * 512].rearrange(
                     "(a two p) c -> p a two c", two=2, p=128))
            obs.append(ob)

        from contextlib import ExitStack as _ES
        stP = _ES()
        psO = stP.enter_context(tc.tile_pool(name="av_ps", bufs=2,
                                             space="PSUM"))
        # p7's remaining score units interleave with the first AVs so
        # neither the PE nor ACT stalls at the phase boundary
        for _ in range(8):
            emit_sunit()
            emit_avs(1)          # heads 0..7 in order


        # ---------------- AV tail + out-projection, interleaved ----------
        # outproj accumulates over kt (= pair index), so ct-group psums can
        # start as soon as early pairs' oT are normalized while the last
        # heads' AV still runs.
        with tc.tile_pool(name="wo_ps", bufs=2, space="PSUM") as psW, \
             tc.tile_pool(name="dn_ps", bufs=2, space="PSUM") as dpsn, \
             tc.tile_pool(name="wband2", bufs=4) as wb2:

            def emit_norm(p):
                # broadcast both heads' 1/den with one K=2 matmul, then
                # scale o^T in place
                dps = dpsn.tile([128, W], F32, tag="dn", name="dnps")
                nc.tensor.matmul(dps[:], selm[:], den2s[p][:])
                nc.vector.tensor_mul(out=o8[:, p, :], in0=oT[p][:],
                                     in1=dps[:])

            # prefetch conv1 weights during the outproj (the sync queue is
            # otherwise idle here and conv1 starts right after)
            wdma(out=c1[:], in_=l1T_d[:].rearrange("(a p) c -> p a c", p=128))

            def op_group(cts, kts, pss, first, last):
                gi = cts[0] // 4
                for kt in kts:
                    for i, ct in enumerate(cts):
                        nc.tensor.matmul(
                            pss[i][:],
                            obs[gi][:, kt // 2, kt % 2,
                                    (ct % 4) * 128:(ct % 4 + 1) * 128],
                            o8[:, kt, :],
                            start=(kt == first), stop=(kt == last))

            # AV pairs 6,7 pulled ahead so their den-reciprocals finish
            # early; the kt accumulation order (0,1,2,3,6,7,4,5) follows
            # norm availability. Two 2-ct psum groups in flight (4 banks).
            avq[:] = [12, 13, 14, 15, 8, 9, 10, 11]
            KTO = (0, 1, 2, 3, 6, 7, 4, 5)
            gA = (0, 1)
            wopA = [psW.tile([128, W], F32, tag="wo", name=f"wopA{i}")
                    for i in range(2)]
            emit_avs(2)          # heads 12,13 -> den p6
            emit_norm(0)
            emit_norm(1)
            op_group(gA, (0, 1), wopA, 0, 5)
            emit_avs(2)          # heads 14,15 -> den p7
            emit_norm(2)
            emit_norm(3)
            op_group(gA, (2, 3), wopA, 0, 5)
            emit_avs(2)          # heads 8,9   -> den p4
            emit_norm(6)
            emit_norm(7)
            op_group(gA, (6, 7), wopA, 0, 5)
            emit_avs(2)          # heads 10,11 -> den p5
            emit_norm(4)
            emit_norm(5)
            op_group(gA, (4, 5), wopA, 0, 5)
            for i, ct in enumerate(gA):
                nc.vector.scalar_tensor_tensor(
                    out=h[ct][:], in0=wopA[i][:], scalar=cinv,
                    in1=h0a[:, ct, :], op0=ALU.mult, op1=ALU.add)
            for gX in ((2, 3), (4, 5), (6, 7)):
                wopX = [psW.tile([128, W], F32, tag="wo",
                                 name=f"wop{gX[0]}_{i}") for i in range(2)]
                op_group(gX, KTO, wopX, 0, 5)
                for i, ct in enumerate(gX):
                    nc.vector.scalar_tensor_tensor(
                        out=h[ct][:], in0=wopX[i][:], scalar=cinv,
                        in1=h0a[:, ct, :], op0=ALU.mult, op1=ALU.add)

        stP.close()  # AV psum pool closes
        stS.close()  # scores psum pool closes

        # attention-phase SBUF is no longer needed; conv buffers take its
        # place in pools opened only now (pools close LIFO, hence the split).
        stA.close()
        stB = ExitStack()
        pC = stB.enter_context(tc.tile_pool(name="pC", bufs=1))
        with (
            tc.tile_pool(name="wband3", bufs=4) as wb3,
            tc.tile_pool(name="conv_ps", bufs=8, space="PSUM") as ps8,
        ):
            # ---------------- conv1 (1x1) + bn1 + relu ----------------
            y1 = pC.tile([128, NT, 528], FP8, tag="y1")
            # l1T was prefetched into c1 (pB) during the out-projection
            c1bands = [c1[:, kt, :] for kt in range(NT)]
            # boundary pre-chain: the two window-edge output columns only,
            # so the halo AllReduce launches long before conv2 tap0/tap2
            # need it
            # one psum tile per mt: a start=True matmul clears its whole
            # PSUM bank, so accumulation groups must not share one
            bps = [ps8.tile([128, 2], F32, tag="ps", name=f"bps{i}")
                   for i in range(NT)]
            for kt in range(NT):
                for mt in range(NT):
                    nc.tensor.matmul(
                        bps[mt][:], c1bands[kt][:, mt * 128:(mt + 1) * 128],
                        h[kt][:, 0:W:W - 1],
                        start=(kt == 0), stop=(kt == NT - 1))
            bc = pm.tile([128, NT, 2], F32, tag="bc")
            for mt in range(NT):
                nc.vector.tensor_scalar(
                    out=bc[:, mt, :], in0=bps[mt][:],
                    scalar1=b1[:, mt:mt + 1], scalar2=0.0,
                    op0=ALU.add, op1=ALU.max)
            cc1i = dp.tile([128, 16], F32, tag="cc1i")
            cc1o = dp.tile([128, 16], F32, tag="cc1o")
            nc.sync.dma_start(out=cc1i[:],
                              in_=bc[:].rearrange("p a b -> p (a b)"))
            nc.gpsimd.collective_compute(
                "AllReduce", ALU.add, replica_groups=RG,
                ins=[cc1i[:].opt()], outs=[cc1o[:].opt()])
            gs = pm.tile([128, NT, 2], F32, tag="gs")
            nc.sync.dma_start(out=gs[:].rearrange("p a b -> p (a b)"),
                              in_=cc1o[:])
            pss = [ps8.tile([128, W], F32, tag="ps", name=f"c1ps{i}")
                   for i in range(NT)]
            for kt in range(NT):
                for mt in range(NT):
                    nc.tensor.matmul(
                        pss[mt][:], c1bands[kt][:, mt * 128:(mt + 1) * 128],
                        h[kt][:],
                        start=(kt == 0), stop=(kt == NT - 1))
            for mt in range(NT):
                nc.scalar.activation(out=y1[:, mt, 1:W + 1], in_=pss[mt][:],
                                     func=AF.Relu, bias=b1[:, mt:mt + 1],
                                     scale=1.0)
            # halo = (gsum . sel) - (own . sel);  sel = mA*left + mB*right
            t1 = pm.tile([128, NT, 1], F32, tag="t1")
            t2 = pm.tile([128, NT, 1], F32, tag="t2")
            halo = pm.tile([128, NT, 1], F32, tag="halo")
            nc.vector.tensor_scalar_mul(out=t1[:], in0=gs[:, :, 0:1], scalar1=mA)
            nc.vector.tensor_scalar_mul(out=t2[:], in0=gs[:, :, 1:2], scalar1=mB)
            nc.vector.tensor_add(out=halo[:], in0=t1[:], in1=t2[:])
            nc.vector.tensor_scalar_mul(out=t1[:], in0=bc[:, :, 0:1], scalar1=mA)
            nc.vector.tensor_scalar_mul(out=t2[:], in0=bc[:, :, 1:2], scalar1=mB)
            nc.vector.tensor_add(out=t1[:], in0=t1[:], in1=t2[:])
            nc.vector.tensor_sub(out=halo[:], in0=halo[:], in1=t1[:])
            # left halo col = halo*mB (zero at the global left edge),
            # right halo col = halo*mA
            for mt in range(NT):
                nc.vector.tensor_scalar_mul(out=y1[:, mt, 0:1],
                                            in0=halo[:, mt, :], scalar1=mB)
                nc.vector.tensor_scalar_mul(out=y1[:, mt, W + 1:W + 2],
                                            in0=halo[:, mt, :], scalar1=mA)

            # ---------------- conv2 (k=3) + bn2 + relu ----------------
            y2 = [pC.tile([128, W], BF16, tag=f"y2_{i}", name=f"y2_{i}")
                  for i in range(NT)]
            pss = [ps8.tile([128, W], F32, tag="ps", name=f"c2ps{i}")
                   for i in range(NT)]
            tap_order = [1, 0, 2]  # halo-free tap first: overlaps the AR
            for ti, tap in enumerate(tap_order):
                c2b = wb3.tile([128, NT // 2, 2, C], FP8, tag="band", bufs=2)
                nc.gpsimd.dma_start(
                    out=c2b[:],
                    in_=l2T_d[tap].rearrange("(a two p) c -> p a two c",
                                             two=2, p=128))
                for a in range(NT // 2):
                    for mt in range(NT):
                        nc.tensor.matmul(
                            pss[mt][:],
                            c2b[:, a, :, mt * 128:(mt + 1) * 128],
                            y1[:, 2 * a:2 * a + 2, tap:tap + W],
                            start=(ti == 0 and a == 0),
                            stop=(ti == 2 and a == NT // 2 - 1),
                            perf_mode=DR)
            for mt in range(NT):
                nc.scalar.activation(out=y2[mt][:], in_=pss[mt][:],
                                     func=AF.Relu, bias=b2[:, mt:mt + 1],
                                     scale=1.0 / 32.0)

            # ------------- conv3 (1x1) + bn3 + residual + stats ----------
            y = pC.tile([128, NT, W], F32, tag="y")
            yp = pC.tile([128, NT, W // 2], F32, tag="yp")
            c3 = wb3.tile([128, NT, C], BF16, tag="c3band", bufs=1)
            nc.gpsimd.dma_start(
                out=c3[:], in_=l3T_d[:].rearrange("(a p) c -> p a c", p=128))
            c3bands = [c3[:, kt, :] for kt in range(NT)]
            st = pm.tile([128, 16], F32, tag="st")
            cc2ia = dp.tile([128, 4], F32, tag="cc2ia")
            cc2oa = dp.tile([128, 4], F32, tag="cc2oa")
            cc2ib = dp.tile([128, 12], F32, tag="cc2ib")
            cc2ob = dp.tile([128, 12], F32, tag="cc2ob")
            # ct-outer so each output tile finishes early and its stats +
            # maxpool (max commutes with the final monotone relu(a*x+b),
            # a=rstd>0) overlap the remaining matmuls. st is (sum, sumsq)
            # pair-interleaved per ct so the stats AllReduce can be split:
            # cts 0..5 launch while cts 6,7 still compute.
            for ct in range(NT):
                psc = ps8.tile([128, W], F32, tag="ps", name=f"c3ps{ct}")
                for kt in range(NT):
                    nc.tensor.matmul(
                        psc[:], c3bands[kt][:, ct * 128:(ct + 1) * 128],
                        y2[kt][:],
                        start=(kt == 0), stop=(kt == NT - 1))
                nc.vector.scalar_tensor_tensor(
                    out=y[:, ct, :], in0=psc[:], scalar=b3[:, ct:ct + 1],
                    in1=h[ct][:], op0=ALU.add, op1=ALU.add)
                nc.vector.reduce_sum(out=st[:, 2 * ct:2 * ct + 1],
                                     in_=y[:, ct, :], axis=AX.X)
                scr = pC.tile([128, W], F32, tag="scr", bufs=2)
                nc.scalar.activation(out=scr[:], in_=y[:, ct, :],
                                     func=AF.Square, scale=1.0 / 32.0,
                                     accum_out=st[:, 2 * ct + 1:2 * ct + 2])
                yv = y[:, ct, :].rearrange("p (l t) -> p l t", t=2)
                nc.vector.tensor_max(out=yp[:, ct, :].unsqueeze(2),
                                     in0=yv[:, :, 0:1], in1=yv[:, :, 1:2])
                if ct == 1:
                    nc.gpsimd.dma_start(out=cc2ia[:], in_=st[:, 0:4])
                    nc.gpsimd.collective_compute(
                        "AllReduce", ALU.add, replica_groups=RG,
                        ins=[cc2ia[:].opt()], outs=[cc2oa[:].opt()])
        # ------------- instance-norm stats + pair AllReduce -------------
        with tc.tile_pool(name="fin_sb", bufs=1) as fsb:
            nc.gpsimd.dma_start(out=cc2ib[:], in_=st[:, 4:16])
            nc.gpsimd.collective_compute(
                "AllReduce", ALU.add, replica_groups=RG,
                ins=[cc2ib[:].opt()], outs=[cc2ob[:].opt()])
            gst = pm.tile([128, 16], F32, tag="gst")
            nc.sync.dma_start(out=gst[:, 0:4], in_=cc2oa[:])
            nc.sync.dma_start(out=gst[:, 4:16], in_=cc2ob[:])

            eps_sb = pm.tile([128, 1], F32, tag="eps_sb")
            nc.vector.memset(eps_sb[:], EPS)
            mean = pm.tile([128, 8], F32, tag="mean")
            ms = pm.tile([128, 8], F32, tag="ms")
            rstd = pm.tile([128, 8], F32, tag="rstd")
            shift = pm.tile([128, 8], F32, tag="shift")
            yo = fsb.tile([128, NT, W // 2], F32, tag="yo")

            def finalize(lo, hi):
                # stats chunk [lo,hi): normalize+relu+store per tile
                nc.vector.tensor_scalar_mul(
                    out=mean[:, lo:hi], in0=gst[:, 2 * lo:2 * hi:2],
                    scalar1=1.0 / L)
                nc.vector.tensor_mul(out=shift[:, lo:hi], in0=mean[:, lo:hi],
                                     in1=mean[:, lo:hi])
                nc.vector.tensor_sub(out=ms[:, lo:hi],
                                     in0=gst[:, 2 * lo + 1:2 * hi:2],
                                     in1=shift[:, lo:hi])
                nc.scalar.activation(out=ms[:, lo:hi], in_=ms[:, lo:hi],
                                     func=AF.Sqrt, bias=eps_sb[:], scale=1.0)
                nc.vector.reciprocal_approx_fast(out=rstd[:, lo:hi],
                                                 in_=ms[:, lo:hi])
                nc.vector.tensor_scalar(out=shift[:, lo:hi],
                                        in0=mean[:, lo:hi],
                                        scalar1=-1.0, scalar2=0.0,
                                        op0=ALU.mult, op1=ALU.add)
                nc.vector.tensor_mul(out=shift[:, lo:hi], in0=shift[:, lo:hi],
                                     in1=rstd[:, lo:hi])
                for ct in range(lo, hi):
                    nc.scalar.activation(
                        out=yo[:, ct, :], in_=yp[:, ct, :], func=AF.Relu,
                        scale=rstd[:, ct:ct + 1], bias=shift[:, ct:ct + 1])
                    nc.scalar.dma_start(
                        out=out_d[:].rearrange(
                            "(a p) l -> p a l", p=128)[:, ct, :],
                        in_=yo[:, ct, :])

            finalize(0, 2)
            finalize(2, 8)
        stB.close()

    nc.compile()
    return nc


_NC = None


def _get_nc():
    global _NC
    if _NC is None:
        _NC = _build()
    return _NC


def _prep_inputs(inputs):
    f = lambda k: np.asarray(inputs[k], dtype=np.float32)
    bf = lambda a: np.ascontiguousarray(a.astype(ml_dtypes.bfloat16))
    x = f("x")

    s0 = f("norm_g") / np.sqrt(f("norm_v") + EPS)
    t0 = f("norm_b") - f("norm_m") * s0

    # q/k/v/o weights x32 in fp8 (0.02-scale weights would sit below the
    # e4m3 min normal); the scale unwinds in the exp ACTIVATE (1/32768)
    # and the residual add (1/1024)
    q8w = lambda a: np.ascontiguousarray(
        (np.asarray(a, np.float32) * 32.0).astype(E4))
    wqT = q8w(f("wq").T)
    wkT = q8w(f("wk").T)
    wvT = q8w(f("wv").T)
    woT = q8w(f("wo").T)

    s1 = f("bn1_g") / np.sqrt(f("bn1_v") + EPS)
    b1 = s1 * (f("cb1") - f("bn1_m")) + f("bn1_b")
    l1T = bf((s1[:, None] * f("cw1")[:, :, 0]).T)

    s2 = f("bn2_g") / np.sqrt(f("bn2_v") + EPS)
    b2 = s2 * (f("cb2") - f("bn2_m")) + f("bn2_b")
    cw2 = f("cw2")
    l2T = np.ascontiguousarray((np.stack(
        [(s2[:, None] * cw2[:, :, k]).T for k in range(3)],
        axis=0) * 32.0).astype(E4))

    s3 = f("bn3_g") / np.sqrt(f("bn3_v") + EPS)
    b3 = s3 * (f("cb3") - f("bn3_m")) + f("bn3_b")
    l3T = bf((s3[:, None] * f("cw3")[:, :, 0]).T)

    selm = np.zeros((2, 128), np.float32)
    selm[0, :DH] = 1.0
    selm[1, DH:] = 1.0

    def cols(v):  # (1024,) -> (128, 8): channel c = col*128 + partition
        return np.ascontiguousarray(v.reshape(8, 128).T.astype(np.float32))

    in_maps = []
    for core in range(8):
        n, half = core // 2, core % 2
        xc = x[n] if half == 0 else np.roll(x[n], -W, axis=1)
        vecs = np.zeros((128, 43), np.float32)
        vecs[:, 42] = 1.0 / 1024.0
        vecs[:, 0:8] = cols(s0)
        vecs[:, 8:16] = cols(t0)
        vecs[:, 16:24] = cols(b1)
        vecs[:, 24:32] = cols(b2)
        vecs[:, 32:40] = cols(b3)
        vecs[:, 40] = 1.0 if half == 0 else 0.0   # mA
        vecs[:, 41] = 0.0 if half == 0 else 1.0   # mB
        in_maps.append({
            "x": bf(xc),
            "wqT": wqT, "wkT": wkT, "wvT": wvT, "woT": woT,
            "l1T": l1T, "l2T": l2T, "l3T": l3T,
            "vecs": vecs, "selm": selm,
        })
    return in_maps


def kernel(**inputs):
    global LAST_RESULTS
    nc = _get_nc()
    in_maps = _prep_inputs(inputs)
    res = bass_utils.run_bass_kernel_spmd(
        nc, in_maps, core_ids=list(range(8)), trace=TRACE)
    LAST_RESULTS = res
    out = np.empty((N_BATCH, C, L // 2), np.float32)
    for core in range(8):
        n, half = core // 2, core % 2
        out[n][:, half * (W // 2):(half + 1) * (W // 2)] = res.results[core]["out"]
    return out

